# revision 34
# baseline (speedup 1.0000x reference)
"""AssemblyClassifier Trainium2 kernel: 8-way batch-parallel across NeuronCores.

Per core (batch b), x = input_seq[b] : (28, 16384, 8) f32 is viewed flat as
[112 partitions = 4*e + t_hi, (t_lo, f)] (a contiguous split, so one full-rate
128-partition DMA per 512-t_lo chunk).  The feature sum runs as a 3-level DVE
add-tree; NaN slots are detected with ACT Is_finite and zeroed by a fused DVE
(min 3e38, mult obsf) op that also casts to bf16.  The assembly fold
G2 = [-scale*Ef | alpha*(1-Ef)].T @ eq_classes is computed on device (host
ships the tiny tables partition-replicated in bf16); per-t_hi masked copies
G2m[g] make K=112 matmuls valid for all four t_hi groups off one lhsT, with
matmul columns t-interleaved (j::4) so each out-DMA partition writes one
contiguous 2KB run.  PSUM -> ACT copy -> bf16 SBUF -> one fused 1MB DMA per
chunk; output is written bf16 and upcast to f32 on host.  ~99 us/NEFF vs a
~91 us f32-IO memory roofline (input 14.7MB + tables + bf16 output 8.4MB per
core at ~300 GB/s effective, plus ~18 us fixed NEFF preamble/drain).
"""
import os
import sys
import types

import numpy as np

_B, _E, _T, _F = 8, 28, 16384, 8
_A, _C = 1024, 256
_HI = 4                 # t_hi groups (partition dim = 4*e + t_hi)
_TL = _T // _HI         # 4096 t_lo per group
_NT = 512               # t_lo chunk
_NJ = _NT // 128        # 128-col groups per chunk
_NCHUNK = _TL // _NT    # 8
_P = _E * _HI           # 112 active partitions

_cache = {}
LAST_RESULTS = None


def _ensure_axon_hooks():
    """The RL image's antenv lacks axon_hooks; shim it so trace=True works."""
    try:
        import antenv.axon_hooks  # noqa: F401
        return
    except Exception:
        pass
    try:
        from trn_agent_boot.trn_boot import _ntff_profile_via_ctypes
        hook = _ntff_profile_via_ctypes('/opt/axon/libaxon_pjrt.so')
    except Exception:
        hook = None
    m = types.ModuleType('antenv.axon_hooks')
    m.get_axon_ntff_profile_hook = lambda: hook
    m.set_axon_ntff_profile_hook = lambda h: None
    sys.modules['antenv.axon_hooks'] = m


def _build():
    import concourse.bass as bass
    import concourse.mybir as mybir
    from concourse import bacc
    from concourse.tile import TileContext

    F32 = mybir.dt.float32
    BF16 = mybir.dt.bfloat16
    ALU = mybir.AluOpType
    ACTF = mybir.ActivationFunctionType

    nc = bacc.Bacc("TRN2", target_bir_lowering=False)
    x = nc.declare_dram_parameter("x", [_E, _T, _F], F32, isOutput=False)
    eq = nc.declare_dram_parameter("eq", [_A, _C], BF16, isOutput=False)
    efp = nc.declare_dram_parameter("efp", [_A, 256], BF16, isOutput=False)
    maskm = nc.declare_dram_parameter("maskm", [128, _HI], F32, isOutput=False)
    out = nc.declare_dram_parameter("out", [_T, _C], BF16, isOutput=True)

    # flat [112, (t_lo f)] view of x; partition p = 4*e + t_hi
    xv = x[:].rearrange("e (hi tl) f -> (e hi) (tl f)", hi=_HI)
    # out view: [(g q), p, j, c] so DMA dst partition dim is p
    ov = out[:].rearrange("(g q p j) c -> q p g (j c)", g=_HI, q=_NCHUNK,
                          p=128, j=_NJ)

    with TileContext(nc) as tc:
        with (
            tc.tile_pool(name="const", bufs=1) as constp,
            tc.tile_pool(name="xin", bufs=6) as xin,
            tc.tile_pool(name="work", bufs=2) as work,
            tc.tile_pool(name="mm", bufs=3) as mmp,
            tc.tile_pool(name="psum", bufs=2, space="PSUM") as psp,
            tc.tile_pool(name="outp", bufs=2) as outp,
        ):
            # ---------------- G2 table prep ----------------
            # eq/efp as bf16 (SWDGE cast-DMA), 8 a-chunks of 128 on partitions.
            # efp host layout: col 4e+r = -scale*Ef[:,e] (cols 0:112) and
            # col 128+4e+r = alpha*(1-Ef[:,e]) -- already partition-replicated.
            eq_sb = constp.tile([128, _A // 128, _C], BF16)
            nc.sync.dma_start(out=eq_sb[:],
                              in_=eq[:].rearrange("(c p) n -> p c n", p=128))
            efp_sb = constp.tile([128, _A // 128, 256], BF16)
            nc.sync.dma_start(out=efp_sb[:],
                              in_=efp[:].rearrange("(c p) n -> p c n", p=128))
            mask_sb = constp.tile([128, _HI], F32)
            nc.sync.dma_start(out=mask_sb[:], in_=maskm[:])

            # prefetch chunk 0 of x right after the small table loads
            xt0 = xin.tile([128, _NT * _F], F32, name="xt")
            nc.sync.dma_start(out=xt0[0:_P, :], in_=xv[:, 0:_NT * _F])

            # psum[k][4e+r, c] = G_k[e, c]; mask to per-group tables on evac
            g2m = [constp.tile([128, 2 * _C], BF16, name=f"g2m{g}")
                   for g in range(_HI)]
            for k in range(2):
                pp = psp.tile([128, 512], mybir.dt.float32, name="pt0")
                for ci in range(_A // 128):
                    nc.tensor.matmul(pp[0:_P, 0:_C],
                                     efp_sb[:, ci, 128 * k:128 * k + _P],
                                     eq_sb[:, ci, :],
                                     start=(ci == 0), stop=(ci == _A // 128 - 1))
                for g in range(_HI):
                    nc.scalar.activation(out=g2m[g][0:_P, k * _C:(k + 1) * _C],
                                         in_=pp[0:_P, 0:_C], func=ACTF.Copy,
                                         scale=mask_sb[0:_P, g:g + 1])

            # ---------------- main pipeline ----------------
            # all 256-col blocks view: t = g*4096 + w*256 + p*2 + j
            ovw = out[:].rearrange("(g w p j) c -> w p g (j c)", g=_HI,
                                   w=_TL // 256, p=128, j=2)

            def process(xt, col0, nt, wslot):
                """F-sum + select + matmuls + evac + out-DMA for nt t_lo cols
                of xt starting at column col0.  wslot indexes 256-col blocks
                when nt == 256 (ovw), else full-_NT chunks (ov)."""
                nj = nt // 128
                x3 = xt[0:_P, col0 * _F:(col0 + nt) * _F].rearrange(
                    "p (tl f) -> p tl f", f=8)
                l1 = work.tile([128, nt * 4], F32, name="l1")
                nc.vector.tensor_tensor(out=l1[0:_P, 0:nt * 4], in0=x3[:, :, 0:4],
                                        in1=x3[:, :, 4:8], op=ALU.add)
                l13 = l1[0:_P, 0:nt * 4].rearrange("p (tl f) -> p tl f", f=4)
                l2 = work.tile([128, nt * 2], F32, name="l2")
                nc.vector.tensor_tensor(out=l2[0:_P, 0:nt * 2], in0=l13[:, :, 0:2],
                                        in1=l13[:, :, 2:4], op=ALU.add)
                l23 = l2[0:_P, 0:nt * 2].rearrange("p (tl f) -> p tl f", f=2)
                s_raw = work.tile([128, nt], F32, name="s_raw")
                nc.vector.tensor_tensor(out=s_raw[0:_P, 0:nt], in0=l23[:, :, 0:1],
                                        in1=l23[:, :, 1:2], op=ALU.add)

                obsf = mmp.tile([128, nt], BF16, name="obsf")
                nc.scalar.activation(out=obsf[0:_P, 0:nt], in_=s_raw[0:_P, 0:nt],
                                     func=ACTF.Is_finite)
                s0 = mmp.tile([128, nt], BF16, name="s0")
                nc.vector.scalar_tensor_tensor(out=s0[0:_P, 0:nt],
                                               in0=s_raw[0:_P, 0:nt],
                                               scalar=3.0e38, in1=obsf[0:_P, 0:nt],
                                               op0=ALU.min, op1=ALU.mult)

                og = outp.tile([128, _HI * nj * _C], BF16, name="og")
                for jp in range(nj // 2):
                    pts = [psp.tile([128, 512], mybir.dt.float32, name=f"pt{g}")
                           for g in range(_HI)]
                    for jj in range(2):
                        j = 2 * jp + jj
                        lhs_s0 = s0[0:_P, j::nj]
                        lhs_ob = obsf[0:_P, j::nj]
                        for g in range(_HI):
                            nc.tensor.matmul(pts[g][:, jj * _C:(jj + 1) * _C],
                                             lhs_s0, g2m[g][0:_P, 0:_C],
                                             start=True, stop=False)
                            nc.tensor.matmul(pts[g][:, jj * _C:(jj + 1) * _C],
                                             lhs_ob, g2m[g][0:_P, _C:2 * _C],
                                             start=False, stop=True)
                    for g in range(_HI):
                        nc.scalar.copy(
                            out=og[:, g * nj * _C + jp * 512:g * nj * _C + (jp + 1) * 512],
                            in_=pts[g][:])
                view = ov if nt == _NT else ovw
                eng = nc.sync if nt == _NT else nc.scalar
                eng.dma_start(out=view[wslot],
                              in_=og[:, 0:_HI * nj * _C].rearrange(
                                  "p (g jc) -> p g jc", g=_HI))

            for ci in range(_NCHUNK):
                if ci == 0:
                    xt = xt0
                else:
                    xt = xin.tile([128, _NT * _F], F32, name="xt")
                    nc.sync.dma_start(out=xt[0:_P, :],
                                      in_=xv[:, ci * _NT * _F:(ci + 1) * _NT * _F])
                if ci < _NCHUNK - 1:
                    process(xt, 0, _NT, ci)
                else:
                    # split the final chunk to shorten the pipeline drain
                    base_w = ci * (_NT // 256)
                    process(xt, 0, 256, base_w)
                    process(xt, 256, 256, base_w + 1)
    nc.compile()
    return nc


def _get_nc():
    if "nc" not in _cache:
        _ensure_axon_hooks()
        from concourse import bass_utils
        bass_utils.upload_artifacts = lambda tmpdir: "local://skipped"
        _cache["nc"] = _build()
    return _cache["nc"]


def kernel(input_seq, eq_classes, scale, alpha, edge_present):
    global LAST_RESULTS
    x = np.asarray(input_seq, dtype=np.float32)
    eqc = np.asarray(eq_classes, dtype=np.float32)
    ef = np.asarray(edge_present).astype(np.float32)
    sc = float(np.asarray(scale))
    al = float(np.asarray(alpha))

    # host-side prep of tiny tables (scale/alpha folded in), pre-cast to bf16
    import ml_dtypes
    efp = np.zeros((_A, 256), np.float32)
    efp[:, 0:_P] = np.repeat(-sc * ef, _HI, axis=1)
    efp[:, 128:128 + _P] = np.repeat(al * (1.0 - ef), _HI, axis=1)
    efp = efp.astype(ml_dtypes.bfloat16)
    eqc = eqc.astype(ml_dtypes.bfloat16)
    maskm = np.zeros((128, _HI), np.float32)
    for g in range(_HI):
        maskm[g::_HI, g] = 1.0
    maskm[_P:, :] = 0.0

    nc = _get_nc()
    from concourse import bass_utils
    in_maps = [{"x": np.ascontiguousarray(x[b]), "eq": eqc, "efp": efp,
                "maskm": maskm} for b in range(_B)]
    trace = bool(os.environ.get("KERNEL_TRACE"))
    res = bass_utils.run_bass_kernel_spmd(nc, in_maps, core_ids=list(range(_B)),
                                          trace=trace)
    LAST_RESULTS = res
    return np.stack([np.asarray(res.results[b]["out"]).astype(np.float32) for b in range(_B)], axis=0)
